# revision 100
# baseline (speedup 1.0000x reference)
"""Trainium2 Bass kernel for a GQA attention block (B=2, S=2048, H=2048,
16 q-heads / 8 kv-heads, head_dim=128, fp32), tensor-parallel over heads
across 8 NeuronCores.

Per-core shard (core c): q-heads {2c, 2c+1}, kv-head c; wq/wk/wv column
shards, wo row shard. x is replicated (pre-transposed to [HID, T] bf16 on
host). Each core emits a partial [4096, 2048] f32 o-proj product; the host
gather sums the 8 partials.

Device dataflow (per core), all matmul inputs bf16:
  A) QKV^T projections ([d, tok] layout): x streamed as [128, 1024] bf16
     chunks (two 512-token tiles per DMA), RoPE tables resident in SBUF.
     Per 512-token tile and head: one ACT copy evicts the PSUM slab;
     RMSNorm sum-of-squares on DVE (bf16) + GPSIMD partition-allreduce
     (result in ALL partitions); rstd = Abs_reciprocal_sqrt on ACT over the
     full tile (no partition broadcast, no DVE reciprocal); RoPE as
     partition-half shuffle matmul; rstd applied after RoPE (commutes).
     V^T slabs are evicted to bf16 and transposed to natural [tok, d]
     via DMA-xbar transpose (no PE transposes).
  B) Causal attention per (batch, q-tile, head, 256-q slice):
     scores S^T [128 k, 256 q] matmuls two-packed per PSUM bank; exp on
     ACT (no max subtraction -- RMSNorm bounds |scores| <= sqrt(128));
     causal masking of the diagonal band via DVE multiply with two static
     0/1 mask tiles; softmax denominator entirely off the PE: DVE bf16
     pair-adds + one GPSIMD partition-allreduce + full-tile reciprocal;
     PV accumulated over k-tiles into a persistent ping-pong PSUM bank.
     (b0,qt0)'s score stage is emitted inside phase A's tail, and o-proj
     runs one q-tile behind the slices, so the PE never waits on the
     exp/denominator chain. o-proj partials are staged to SBUF as bf16
     via paired [128, 1024] evictions alternating DVE/ACT, then one
     row-contiguous DMA per 128-token block; the host sums bf16 partials
     in f64.
"""

import math
import os
import sys

import numpy as np

for _p in ("/opt/trn_rl_repo", "/root/.axon_site/_ro/trn_rl_repo"):
    if os.path.isdir(_p) and _p not in sys.path:
        sys.path.insert(0, _p)
        break

import concourse.bacc as bacc
import concourse.tile as tile
from concourse import mybir
from concourse.bass_isa import ReduceOp
from concourse.bass_utils import run_bass_kernel_spmd

# Problem constants (hardcoded per contract)
B, S, HID = 2, 2048, 2048
NH, NKV, D = 16, 8, 128
NCORES = 8
HQ = NH // NCORES  # q heads per core = 2
T = B * S          # 4096 tokens
EPS = 1e-5
F32 = mybir.dt.float32
BF16 = mybir.dt.bfloat16
MDT = BF16
SCALE = 1.0 / math.sqrt(D)
# rstd on ACT via Abs_reciprocal_sqrt (1/sqrt(|x|), exact for x>=0); set to
# 0 to fall back to Sqrt + DVE reciprocal if HW accuracy disappoints
USE_ARS = os.environ.get("BASS_ARS", "1") == "1"

KT = HID // 128      # 16 contraction tiles
TT = T // 512        # 8 token tiles of 512
QT_PER_B = S // 512  # 4 q-tiles per batch


def build_nc():
    nc = bacc.Bacc("TRN2", target_bir_lowering=False, debug=False)
    xt = nc.dram_tensor("xt", [HID, T], MDT, kind="ExternalInput").ap()
    wqkv = nc.dram_tensor("wqkv", [HID, 4 * D], MDT, kind="ExternalInput").ap()
    woc = nc.dram_tensor("woc", [HQ * D, HID], MDT, kind="ExternalInput").ap()
    pmat = nc.dram_tensor("pmat", [D, D], MDT, kind="ExternalInput").ap()
    # 4 RoPE tables (q-cos, q-sin, k-cos, k-sin), norm weights folded in
    tab4 = nc.dram_tensor("tab4", [D, 4, S], MDT, kind="ExternalInput").ap()
    # partials are summed across cores on the host; bf16 partials keep the
    # final error ~0.4% of partial RMS, well inside the 2e-2 budget
    out = nc.dram_tensor("out", [T, HID], MDT, kind="ExternalOutput").ap()

    with tile.TileContext(nc) as tc:
        from contextlib import ExitStack

        with ExitStack() as root:
            const = root.enter_context(tc.tile_pool(name="const", bufs=1))
            pmat_sb = const.tile([D, D], MDT, name="pmat_sb")
            eps_col = const.tile([128, 1], F32, name="eps_col")
            nc.vector.memset(eps_col, EPS)
            # causal masks for the two diagonal-band k-tiles of a 256-q
            # slice: cmask[:, j, q] = 1 iff q >= p + 128*j. Applied as a DVE
            # multiply (cheaper + off the Pool critical path vs affine_select)
            cmask = const.tile([128, 2, 256], MDT, name="cmask")
            nc.vector.memset(cmask, 1.0)
            for j in range(2):
                nc.gpsimd.affine_select(
                    out=cmask[:, j, :], in_=cmask[:, j, :],
                    pattern=[[1, 256]], channel_multiplier=-1, base=-128 * j,
                    compare_op=mybir.AluOpType.is_ge, fill=0.0,
                )

            res = root.enter_context(tc.tile_pool(name="res", bufs=1))
            wo_sb = res.tile([128, HQ, HID], MDT, name="wo_sb")
            qt_sb = res.tile([128, HQ, T], MDT, name="qt_sb")   # [d, h, tok]
            kt_sb = res.tile([128, T], MDT, name="kt_sb")       # [d, tok]
            v_sb = res.tile([128, T // 128, D], MDT, name="v_sb")  # [tok%128, tile, d]
            tab_sb = res.tile([128, 4, S], MDT, name="tab_sb")

            # attention score-stage pools live at root (PSUM banks 0-2) so
            # (b0,qt0)'s scores can be emitted inside phase A's tail
            ep = root.enter_context(tc.tile_pool(name="ep", bufs=20))
            wp2 = root.enter_context(tc.tile_pool(name="wp2", bufs=9))
            rbp = root.enter_context(tc.tile_pool(name="rbp", bufs=14))
            psS = root.enter_context(tc.tile_pool(name="psS", bufs=3, space="PSUM"))
            psO = root.enter_context(tc.tile_pool(name="psO", bufs=1, space="PSUM"))
            # one persistent PV bank, halves ping-ponged by slice parity
            psO_t = psO.tile([128, 2, 256], F32, name="ot2")
            pv_count = [0]
            # 1x1 dummy matmul at t~0: starts the PE pstate ramp while the
            # first weight/x DMAs are in flight (result overwritten by the
            # first PV accumulation's start=True)
            nc.tensor.matmul(psO_t[0:1, 0, 0:1], lhsT=eps_col, rhs=eps_col[:, 0:1],
                             start=True, stop=True)

            def slice_scores(b, q0, h, qh, filler=None):
                # scores, two k-tiles packed per PSUM bank, one exp per
                # pair, causal mask, then the softmax denominator off the
                # PE: pair-sums + chain on DVE, partition allreduce on Pool.
                # `filler` (generator) emits one deferred o-proj PSUM-pair
                # after each score pack: the alternation relaxes both the
                # psP-rotation latency and the exp drain.
                qq0 = q0 + qh * 256
                n_kt = (qq0 + 256) // 128  # valid k tiles
                ets = [None] * n_kt
                etps = []
                for kp in range(n_kt // 2):
                    st = psS.tile([128, 2, 256], F32, name="st", tag="st")
                    for j in range(2):
                        kt = 2 * kp + j
                        nc.tensor.matmul(
                            st[:, j, :],
                            lhsT=(kt_sb[:, b * S + kt * 128: b * S + (kt + 1) * 128]),
                            rhs=(qt_sb[:, h, b * S + qq0: b * S + qq0 + 256]),
                            start=True, stop=True,
                        )
                    etp = ep.tile([128, 2, 256], MDT, name="et", tag="et")
                    nc.scalar.activation(
                        etp, st, mybir.ActivationFunctionType.Exp,
                        scale=SCALE,
                    )
                    if kp == n_kt // 2 - 1:
                        # last pack = the two diagonal-band k-tiles, whose
                        # masks are cmask's two halves in matching order:
                        # one fused [128, 512] multiply
                        nc.vector.tensor_mul(etp, etp, cmask)
                    for j in range(2):
                        ets[2 * kp + j] = etp[:, j, :]
                    etps.append(etp)
                    if filler is not None:
                        next(filler, None)
                # balanced reduction tree (depth ~log2) instead of a serial
                # chain: each exp gates only its own leaf, and the at-mul
                # fires ~1us earlier on the biggest slices
                lvl = []
                for etp in etps:
                    s = wp2.tile([128, 256], MDT, name="acc", tag="acc")
                    nc.vector.tensor_add(s, etp[:, 0, :], etp[:, 1, :])
                    lvl.append(s)
                while len(lvl) > 1:
                    nxt = []
                    for i in range(0, len(lvl) - 1, 2):
                        nc.vector.tensor_add(lvl[i], lvl[i], lvl[i + 1])
                        nxt.append(lvl[i])
                    if len(lvl) % 2:
                        nxt.append(lvl[-1])
                    lvl = nxt
                acc = lvl[0]
                nc.gpsimd.partition_all_reduce(acc, acc, 128, ReduceOp.add)
                rb = rbp.tile([128, 256], MDT, name="rb", tag="rb")
                with nc.allow_low_precision(reason="1/den bf16"):
                    nc.vector.reciprocal(rb, acc)
                return (b, h, qh, n_kt, ets, rb)

            b_slices = [(h, qh) for h in range(HQ) for qh in range(2)]
            prestates = {}

            # ---------------- Phase A: QKV^T, norm, rope, V transpose ---------
            with ExitStack() as pa:
                wqp = pa.enter_context(tc.tile_pool(name="wqp", bufs=1))
                xp = pa.enter_context(tc.tile_pool(name="xp", bufs=25))
                xp5 = pa.enter_context(tc.tile_pool(name="xp5", bufs=17))
                wp = pa.enter_context(tc.tile_pool(name="wp", bufs=2))
                psR = pa.enter_context(tc.tile_pool(name="psR", bufs=1, space="PSUM"))
                psA = pa.enter_context(tc.tile_pool(name="psA", bufs=3, space="PSUM"))

                wqkv_sb = wqp.tile([128, KT, 4 * D], MDT, name="wqkv_sb")

                # x streaming. The first two 512-token tiles are loaded as
                # individual [128, 512] slices so the DMA engine keeps pace
                # with the PE during warmup (RoPE table quarters slot into
                # tile 1's stream); later tiles use [128, 1024] chunks.
                xviews = {}  # t -> per-k list of (tile, base_offset)

                def load_wqkv(k0, nk):
                    nc.sync.dma_start(
                        out=wqkv_sb[:, k0:k0 + nk, :],
                        in_=wqkv[k0 * 128:(k0 + nk) * 128, :].rearrange(
                            "(a p) n -> p a n", p=128),
                    )

                def load_tile_split(t):
                    # two k-slices per DMA keeps the HWDGE issue rate
                    # (625ns/DMA, globally serialized) ahead of the PE;
                    # the very first wqkv/x transfers are single-k so the
                    # first matmul's inputs land as early as possible
                    lst = []
                    for kp in range(KT // 2):
                        k = 2 * kp
                        if t == 0:
                            if kp == 0:
                                load_wqkv(0, 2)
                            elif kp == 1:
                                load_wqkv(2, 2)
                            elif kp % 2 == 0:
                                load_wqkv(2 * kp, 4)
                        xk = xp5.tile([128, 2, 512], MDT, name="xk5", tag="xk5")
                        nc.sync.dma_start(
                            out=xk,
                            in_=xt[k * 128:(k + 2) * 128, t * 512:(t + 1) * 512].rearrange(
                                "(a p) n -> p a n", p=128),
                        )
                        lst.append((xk[:, 0, :], 0))
                        lst.append((xk[:, 1, :], 0))
                        if t == 1 and kp % 2 == 1:
                            q = kp // 2
                            nc.sync.dma_start(out=tab_sb[:, q, :], in_=tab4[:, q, :])
                    xviews[t] = lst

                def load_chunk(c):  # tiles 2c, 2c+1 (c >= 1)
                    lst = []
                    for k in range(KT):
                        xk = xp.tile([128, 1024], MDT, name="xk", tag="xk")
                        nc.sync.dma_start(
                            out=xk, in_=xt[k * 128:(k + 1) * 128, c * 1024:(c + 1) * 1024]
                        )
                        lst.append(xk)
                    xviews[2 * c] = [(xk, 0) for xk in lst]
                    xviews[2 * c + 1] = [(xk, 512) for xk in lst]

                load_tile_split(0)
                # pmat off the critical first HWDGE slots; needed at ~15us
                nc.scalar.dma_start(out=pmat_sb, in_=pmat)
                load_tile_split(1)

                for t in range(TT):
                    if t % 2 == 0 and t + 2 < TT:
                        load_chunk(t // 2 + 1)
                    if t == 2:  # wo is not needed until phase B
                        nc.sync.dma_start(
                            out=wo_sb, in_=woc.rearrange("(h p) n -> p h n", p=128)
                        )
                    # four single-bank PSUM accumulators (q0, q1, k, v) on a
                    # 3-buf rotation
                    slabs = []
                    for m in range(4):
                        ps = psA.tile([128, 512], F32, name="ps_qkv", tag="ps_qkv")
                        for k in range(KT):
                            xk, base = xviews[t][k]
                            nc.tensor.matmul(
                                ps,
                                lhsT=(wqkv_sb[:, k, m * 128:(m + 1) * 128]),
                                rhs=(xk[:, base:base + 512]),
                                start=(k == 0),
                                stop=(k == KT - 1),
                            )
                        slabs.append(ps)

                    s0 = (t % QT_PER_B) * 512  # position-in-sequence
                    for m in range(3):  # q0, q1, k
                        ti = 0 if m < 2 else 2  # cos table index (q vs k)
                        cosT = tab_sb[:, ti, s0:s0 + 512]
                        sinT = tab_sb[:, ti + 1, s0:s0 + 512]
                        src = slabs[m]
                        qk = wp.tile([128, 512], MDT, name="qk", tag="qk")
                        nc.scalar.copy(qk, src)  # PSUM eviction (ACT)
                        sq = wp.tile([128, 512], MDT, name="sq", tag="sq")
                        nc.vector.tensor_mul(sq, qk, qk)
                        nc.gpsimd.partition_all_reduce(sq, sq, 128, ReduceOp.add)
                        # rstd in every partition (allreduce output is
                        # replicated): no broadcast needed
                        rstd = wp.tile([128, 512], MDT, name="rstd", tag="rstd")
                        if USE_ARS:
                            nc.scalar.activation(
                                rstd, sq,
                                mybir.ActivationFunctionType.Abs_reciprocal_sqrt,
                                bias=eps_col, scale=1.0 / D,
                            )
                        else:
                            rr = wp.tile([128, 512], F32, name="rr", tag="rr")
                            nc.scalar.activation(
                                rr, sq, mybir.ActivationFunctionType.Sqrt,
                                bias=eps_col, scale=1.0 / D,
                            )
                            with nc.allow_low_precision(reason="rstd bf16"):
                                nc.vector.reciprocal(rstd, rr)
                        shf = psR.tile([128, 512], F32, name="shf", tag="shf")
                        nc.tensor.matmul(shf, lhsT=pmat_sb, rhs=qk, start=True, stop=True)
                        t0 = wp.tile([128, 512], MDT, name="t0", tag="t0")
                        nc.vector.tensor_mul(t0, qk, cosT)
                        t1 = wp.tile([128, 512], MDT, name="t1", tag="t1")
                        nc.vector.tensor_mul(t1, shf, sinT)  # reads PSUM
                        tr = wp.tile([128, 512], MDT, name="tr", tag="tr")
                        nc.vector.tensor_add(tr, t0, t1)
                        if m < 2:
                            dst = qt_sb[:, m, t * 512:(t + 1) * 512]
                        else:
                            dst = kt_sb[:, t * 512:(t + 1) * 512]
                        nc.vector.tensor_mul(dst, tr, rstd)
                    # V: evict transposed VT (bf16) then DMA-xbar transpose
                    # to natural [tok, d]
                    vt = wp.tile([128, 512], MDT, name="vt", tag="vt")
                    nc.scalar.copy(vt, slabs[3])
                    nc.sync.dma_start_transpose(
                        v_sb[:, t * 4:(t + 1) * 4, :], vt
                    )
                    if t == 4:
                        # (b0,qt0)'s attention score stage rides phase A's
                        # tail (its K/Q inputs completed with tile 3)
                        prestates[0] = [
                            slice_scores(0, 0, h, qh) for h, qh in b_slices]

            # ---------------- Phase B: causal attention + o-proj --------------
            with ExitStack() as pb:
                atp = pb.enter_context(tc.tile_pool(name="atp", bufs=8))
                op = pb.enter_context(tc.tile_pool(name="op", bufs=3))
                psP = pb.enter_context(tc.tile_pool(name="psP", bufs=2, space="PSUM"))

                def emit_oproj_mq(b, q0, at_tiles, mq, fine=False):
                    # one 128-row block of the o-proj partial for rows
                    # [b*S+q0, +512): two-bank po pairs so each eviction
                    # moves [128, 1024] (eviction is the o-proj bottleneck
                    # at [128, 512] granularity); one bf16 DMA per block.
                    # fine=True (very last block): halves evicted on DVE+ACT
                    # in parallel and two half-DMAs, shortening the
                    # end-of-program drain chain
                    qh = mq // 2
                    mq2 = mq % 2  # 128-slice within the 256 at tile
                    ob4 = op.tile([128, 4, 512], MDT, name="ob4", tag="ob4")
                    for np_ in range(2):
                        po = psP.tile([128, 2, 512], F32, name="po", tag="po")
                        for j in range(2):
                            nn = 2 * np_ + j
                            for h in range(HQ):
                                nc.tensor.matmul(
                                    po[:, j, :],
                                    lhsT=(at_tiles[(h, qh)][:, mq2 * 128:(mq2 + 1) * 128]),
                                    rhs=(wo_sb[:, h, nn * 512:(nn + 1) * 512]),
                                    start=(h == 0), stop=(h == HQ - 1),
                                )
                        dst = ob4[:, 2 * np_:2 * np_ + 2, :]
                        # GPSIMD cannot read PSUM on HW: DVE/ACT alternate
                        if fine:
                            nc.vector.tensor_copy(ob4[:, 2 * np_, :], po[:, 0, :])
                            nc.scalar.copy(ob4[:, 2 * np_ + 1, :], po[:, 1, :])
                            nc.sync.dma_start(
                                out=out[b * S + q0 + mq * 128:
                                        b * S + q0 + (mq + 1) * 128,
                                        np_ * 1024:(np_ + 1) * 1024],
                                in_=dst,
                            )
                        elif (mq * 2 + np_) % 2 == 0:
                            nc.vector.tensor_copy(dst, po)
                        else:
                            nc.scalar.copy(dst, po)
                    if not fine:
                        nc.sync.dma_start(
                            out=out[b * S + q0 + mq * 128: b * S + q0 + (mq + 1) * 128, :],
                            in_=ob4,
                        )

                def slice_pv(state, at_tiles):
                    # PV accumulation over k-tiles, then normalize
                    b, h, qh, n_kt, ets, rb = state
                    ot = psO_t[:, pv_count[0] % 2, :]
                    pv_count[0] += 1
                    for kt in range(n_kt):
                        nc.tensor.matmul(
                            ot, lhsT=(v_sb[:, b * (S // 128) + kt, :]),
                            rhs=(ets[kt]),
                            start=(kt == 0), stop=(kt == n_kt - 1),
                        )
                    at = atp.tile([128, 256], MDT, name="at", tag="at")
                    nc.vector.tensor_mul(at, ot, rb)
                    at_tiles[(h, qh)] = at

                # o-proj runs one q-tile behind the attention slices so the
                # PE never waits on the softmax-denominator tail
                pending = None
                for b in range(B):
                    for qt in range(QT_PER_B):
                        q0 = qt * 512
                        at_tiles = {}
                        if b == 0 and qt in prestates:
                            # score stage already emitted in phase A's tail
                            for st in prestates[qt]:
                                slice_pv(st, at_tiles)
                        else:
                            for h, qh in b_slices:
                                st = slice_scores(b, q0, h, qh)
                                slice_pv(st, at_tiles)
                        if pending is not None:
                            for mq in range(4):
                                emit_oproj_mq(*pending, mq)
                        pending = (b, q0, at_tiles)
                for mq in range(4):
                    emit_oproj_mq(*pending, mq, fine=(mq == 3))
    nc.compile()
    return nc


def _rot_half(w):
    return np.concatenate([w[D // 2:], w[:D // 2]])


def prep_inputs(x, cos, sin, wq, wk, wv, wo, q_norm_w, k_norm_w):
    """Host-side sharding/layout prep. Returns per-core in_maps."""
    import ml_dtypes
    f = np.float32
    mf = np.dtype(ml_dtypes.bfloat16)
    cvt = lambda a: np.ascontiguousarray(a.astype(mf))
    x = np.asarray(x, f)
    cos = np.asarray(cos, f)
    sin = np.asarray(sin, f)
    wq, wk, wv, wo = (np.asarray(a, f) for a in (wq, wk, wv, wo))
    q_norm_w = np.asarray(q_norm_w, f)
    k_norm_w = np.asarray(k_norm_w, f)

    xt = np.ascontiguousarray(x.reshape(T, HID).T)  # [HID, T]
    ctq = cos.T * q_norm_w[:, None]
    stq = sin.T * _rot_half(q_norm_w)[:, None]
    ctk = cos.T * k_norm_w[:, None]
    stk = sin.T * _rot_half(k_norm_w)[:, None]
    tab4 = np.stack([ctq, stq, ctk, stk], axis=1)  # [D, 4, S]
    # rotate-half permutation (with sign) as a matmul stationary operand:
    # out[d] = sum_j pmat[j, d] * q[j] = sign(d) * q[(d+64) % 128]
    pmat = np.zeros((D, D), f)
    for d in range(D // 2):
        pmat[d + D // 2, d] = -1.0
    for d in range(D // 2, D):
        pmat[d - D // 2, d] = 1.0
    xt_m, tab4_m, pmat_m = cvt(xt), cvt(tab4), cvt(pmat)

    in_maps = []
    for c in range(NCORES):
        wqkv_c = np.ascontiguousarray(np.concatenate([
            wq[:, c * HQ * D:(c + 1) * HQ * D],
            wk[:, c * D:(c + 1) * D],
            wv[:, c * D:(c + 1) * D],
        ], axis=1))
        woc = np.ascontiguousarray(wo[c * HQ * D:(c + 1) * HQ * D, :])
        in_maps.append({
            "xt": xt_m, "wqkv": cvt(wqkv_c), "woc": cvt(woc),
            "pmat": pmat_m, "tab4": tab4_m,
        })
    return in_maps


_NC = None


def get_nc():
    global _NC
    if _NC is None:
        _NC = build_nc()
    return _NC


def kernel(x, cos, sin, wq, wk, wv, wo, q_norm_w, k_norm_w):
    nc = get_nc()
    in_maps = prep_inputs(x, cos, sin, wq, wk, wv, wo, q_norm_w, k_norm_w)
    res = run_bass_kernel_spmd(nc, in_maps, core_ids=list(range(NCORES)))
    acc = np.zeros((T, HID), dtype=np.float64)
    for c in range(NCORES):
        acc += res.results[c]["out"]
    return acc.astype(np.float32).reshape(B, S, HID)


# revision 119
# speedup vs baseline: 1.0004x; 1.0004x over previous
"""Trainium2 Bass kernel for a GQA attention block (B=2, S=2048, H=2048,
16 q-heads / 8 kv-heads, head_dim=128, fp32), tensor-parallel over heads
across 8 NeuronCores.

Per-core shard (core c): q-heads {2c, 2c+1}, kv-head c; wq/wk/wv column
shards, wo row shard. x is replicated (pre-transposed to [HID, T] bf16 on
host). Each core emits a partial [4096, 2048] f32 o-proj product; the host
gather sums the 8 partials.

Device dataflow (per core), all matmul inputs bf16:
  A) QKV^T projections ([d, tok] layout): x streamed as [128, 1024] bf16
     chunks (two 512-token tiles per DMA), RoPE tables resident in SBUF.
     Per 512-token tile and head: one ACT copy evicts the PSUM slab;
     RMSNorm sum-of-squares on DVE (bf16) + GPSIMD partition-allreduce
     (result in ALL partitions); rstd = Abs_reciprocal_sqrt on ACT over the
     full tile (no partition broadcast, no DVE reciprocal); RoPE as
     partition-half shuffle matmul; rstd applied after RoPE (commutes).
     V^T slabs are evicted to bf16 and transposed to natural [tok, d]
     via DMA-xbar transpose (no PE transposes).
  B) Causal attention per (batch, q-tile, head, 256-q slice):
     scores S^T [128 k, 256 q] matmuls two-packed per PSUM bank; exp on
     ACT (no max subtraction -- RMSNorm bounds |scores| <= sqrt(128));
     causal masking of the diagonal band via DVE multiply with two static
     0/1 mask tiles; softmax denominator entirely off the PE: DVE bf16
     pair-adds + one GPSIMD partition-allreduce + full-tile reciprocal;
     PV accumulated over k-tiles into a persistent ping-pong PSUM bank.
     (b0,qt0)'s score stage is emitted inside phase A's tail, and o-proj
     runs one q-tile behind the slices, so the PE never waits on the
     exp/denominator chain. o-proj partials are staged to SBUF as bf16
     via paired [128, 1024] evictions alternating DVE/ACT, then one
     row-contiguous DMA per 128-token block; the host sums bf16 partials
     in f64.
"""

import math
import os
import sys

import numpy as np

for _p in ("/opt/trn_rl_repo", "/root/.axon_site/_ro/trn_rl_repo"):
    if os.path.isdir(_p) and _p not in sys.path:
        sys.path.insert(0, _p)
        break

import concourse.bacc as bacc
import concourse.tile as tile
from concourse import mybir
from concourse.bass_isa import ReduceOp
from concourse.bass_utils import run_bass_kernel_spmd

# Problem constants (hardcoded per contract)
B, S, HID = 2, 2048, 2048
NH, NKV, D = 16, 8, 128
NCORES = 8
HQ = NH // NCORES  # q heads per core = 2
T = B * S          # 4096 tokens
EPS = 1e-5
F32 = mybir.dt.float32
BF16 = mybir.dt.bfloat16
MDT = BF16
SCALE = 1.0 / math.sqrt(D)
# rstd on ACT via Abs_reciprocal_sqrt (1/sqrt(|x|), exact for x>=0); set to
# 0 to fall back to Sqrt + DVE reciprocal if HW accuracy disappoints
USE_ARS = os.environ.get("BASS_ARS", "1") == "1"

KT = HID // 128      # 16 contraction tiles
TT = T // 512        # 8 token tiles of 512
QT_PER_B = S // 512  # 4 q-tiles per batch


def build_nc():
    nc = bacc.Bacc("TRN2", target_bir_lowering=False, debug=False)
    xt = nc.dram_tensor("xt", [HID, T], MDT, kind="ExternalInput").ap()
    wqkv = nc.dram_tensor("wqkv", [HID, 4 * D], MDT, kind="ExternalInput").ap()
    woc = nc.dram_tensor("woc", [HQ * D, HID], MDT, kind="ExternalInput").ap()
    pmat = nc.dram_tensor("pmat", [D, D], MDT, kind="ExternalInput").ap()
    # 4 RoPE tables (q-cos, q-sin, k-cos, k-sin), norm weights folded in
    tab4 = nc.dram_tensor("tab4", [D, 4, S], MDT, kind="ExternalInput").ap()
    # partials are summed across cores on the host; bf16 partials keep the
    # final error ~0.4% of partial RMS, well inside the 2e-2 budget
    out = nc.dram_tensor("out", [T, HID], MDT, kind="ExternalOutput").ap()

    with tile.TileContext(nc) as tc:
        from contextlib import ExitStack

        with ExitStack() as root:
            const = root.enter_context(tc.tile_pool(name="const", bufs=1))
            pmat_sb = const.tile([D, D], MDT, name="pmat_sb")
            eps_col = const.tile([128, 1], F32, name="eps_col")
            nc.vector.memset(eps_col, EPS)
            # causal masks for the two diagonal-band k-tiles of a 256-q
            # slice: cmask[:, j, q] = 1 iff q >= p + 128*j. Applied as a DVE
            # multiply (cheaper + off the Pool critical path vs affine_select)
            cmask = const.tile([128, 2, 256], MDT, name="cmask")
            nc.vector.memset(cmask, 1.0)
            for j in range(2):
                nc.gpsimd.affine_select(
                    out=cmask[:, j, :], in_=cmask[:, j, :],
                    pattern=[[1, 256]], channel_multiplier=-1, base=-128 * j,
                    compare_op=mybir.AluOpType.is_ge, fill=0.0,
                )

            res = root.enter_context(tc.tile_pool(name="res", bufs=1))
            wo_sb = res.tile([128, HQ, HID], MDT, name="wo_sb")
            qt_sb = res.tile([128, HQ, T], MDT, name="qt_sb")   # [d, h, tok]
            kt_sb = res.tile([128, T], MDT, name="kt_sb")       # [d, tok]
            v_sb = res.tile([128, T // 128, D], MDT, name="v_sb")  # [tok%128, tile, d]
            tab_sb = res.tile([128, 4, S], MDT, name="tab_sb")

            # attention score-stage pools live at root (PSUM banks 0-2) so
            # (b0,qt0)'s scores can be emitted inside phase A's tail
            ep = root.enter_context(tc.tile_pool(name="ep", bufs=20))
            wp2 = root.enter_context(tc.tile_pool(name="wp2", bufs=9))
            rbp = root.enter_context(tc.tile_pool(name="rbp", bufs=14))
            psS = root.enter_context(tc.tile_pool(name="psS", bufs=3, space="PSUM"))
            psO = root.enter_context(tc.tile_pool(name="psO", bufs=1, space="PSUM"))
            # one persistent PV bank, halves ping-ponged by slice parity
            psO_t = psO.tile([128, 2, 256], F32, name="ot2")
            pv_count = [0]
            # 1x1 dummy matmul at t~0: starts the PE pstate ramp while the
            # first weight/x DMAs are in flight (result overwritten by the
            # first PV accumulation's start=True)
            nc.tensor.matmul(psO_t[0:1, 0, 0:1], lhsT=eps_col, rhs=eps_col[:, 0:1],
                             start=True, stop=True)

            def slice_scores(b, q0, h, qh, filler=None):
                # scores, two k-tiles packed per PSUM bank, one exp per
                # pair, causal mask, then the softmax denominator off the
                # PE: pair-sums + chain on DVE, partition allreduce on Pool.
                # `filler` (generator) emits one deferred o-proj PSUM-pair
                # after each score pack: the alternation relaxes both the
                # psP-rotation latency and the exp drain.
                qq0 = q0 + qh * 256
                n_kt = (qq0 + 256) // 128  # valid k tiles
                ets = [None] * n_kt
                etps = []
                for kp in range(n_kt // 2):
                    st = psS.tile([128, 2, 256], F32, name="st", tag="st")
                    for j in range(2):
                        kt = 2 * kp + j
                        nc.tensor.matmul(
                            st[:, j, :],
                            lhsT=(kt_sb[:, b * S + kt * 128: b * S + (kt + 1) * 128]),
                            rhs=(qt_sb[:, h, b * S + qq0: b * S + qq0 + 256]),
                            start=True, stop=True,
                        )
                    etp = ep.tile([128, 2, 256], MDT, name="et", tag="et")
                    nc.scalar.activation(
                        etp, st, mybir.ActivationFunctionType.Exp,
                        scale=SCALE,
                    )
                    if kp == n_kt // 2 - 1:
                        # last pack = the two diagonal-band k-tiles, whose
                        # masks are cmask's two halves in matching order:
                        # one fused [128, 512] multiply
                        nc.vector.tensor_mul(etp, etp, cmask)
                    for j in range(2):
                        ets[2 * kp + j] = etp[:, j, :]
                    etps.append(etp)
                    if filler is not None:
                        next(filler, None)
                # balanced reduction tree (depth ~log2) instead of a serial
                # chain: each exp gates only its own leaf, and the at-mul
                # fires ~1us earlier on the biggest slices
                lvl = []
                for etp in etps:
                    s = wp2.tile([128, 256], MDT, name="acc", tag="acc")
                    nc.vector.tensor_add(s, etp[:, 0, :], etp[:, 1, :])
                    lvl.append(s)
                while len(lvl) > 1:
                    nxt = []
                    for i in range(0, len(lvl) - 1, 2):
                        nc.vector.tensor_add(lvl[i], lvl[i], lvl[i + 1])
                        nxt.append(lvl[i])
                    if len(lvl) % 2:
                        nxt.append(lvl[-1])
                    lvl = nxt
                acc = lvl[0]
                nc.gpsimd.partition_all_reduce(acc, acc, 128, ReduceOp.add)
                rb = rbp.tile([128, 256], MDT, name="rb", tag="rb")
                with nc.allow_low_precision(reason="1/den bf16"):
                    nc.vector.reciprocal(rb, acc)
                return (b, h, qh, n_kt, ets, rb)

            b_slices = [(h, qh) for h in range(HQ) for qh in range(2)]
            prestates = {}

            # ---------------- Phase A: QKV^T, norm, rope, V transpose ---------
            with ExitStack() as pa:
                wqp = pa.enter_context(tc.tile_pool(name="wqp", bufs=1))
                xp = pa.enter_context(tc.tile_pool(name="xp", bufs=25))
                xp5 = pa.enter_context(tc.tile_pool(name="xp5", bufs=17))
                wp = pa.enter_context(tc.tile_pool(name="wp", bufs=2))
                psR = pa.enter_context(tc.tile_pool(name="psR", bufs=1, space="PSUM"))
                psA = pa.enter_context(tc.tile_pool(name="psA", bufs=3, space="PSUM"))

                wqkv_sb = wqp.tile([128, KT, 4 * D], MDT, name="wqkv_sb")

                # x streaming. The first two 512-token tiles are loaded as
                # individual [128, 512] slices so the DMA engine keeps pace
                # with the PE during warmup (RoPE table quarters slot into
                # tile 1's stream); later tiles use [128, 1024] chunks.
                xviews = {}  # t -> per-k list of (tile, base_offset)

                def load_wqkv(k0, nk):
                    nc.sync.dma_start(
                        out=wqkv_sb[:, k0:k0 + nk, :],
                        in_=wqkv[k0 * 128:(k0 + nk) * 128, :].rearrange(
                            "(a p) n -> p a n", p=128),
                    )

                def load_tile_split(t):
                    # two k-slices per DMA keeps the HWDGE issue rate
                    # (625ns/DMA, globally serialized) ahead of the PE;
                    # the very first wqkv/x transfers are single-k so the
                    # first matmul's inputs land as early as possible
                    lst = []
                    for kp in range(KT // 2):
                        k = 2 * kp
                        if t == 0:
                            if kp == 0:
                                load_wqkv(0, 2)
                            elif kp == 1:
                                load_wqkv(2, 2)
                            elif kp % 2 == 0:
                                load_wqkv(2 * kp, 4)
                        xk = xp5.tile([128, 2, 512], MDT, name="xk5", tag="xk5")
                        nc.sync.dma_start(
                            out=xk,
                            in_=xt[k * 128:(k + 2) * 128, t * 512:(t + 1) * 512].rearrange(
                                "(a p) n -> p a n", p=128),
                        )
                        lst.append((xk[:, 0, :], 0))
                        lst.append((xk[:, 1, :], 0))
                        if t == 1 and kp % 2 == 1:
                            q = kp // 2
                            nc.sync.dma_start(out=tab_sb[:, q, :], in_=tab4[:, q, :])
                    xviews[t] = lst

                def load_chunk(c):  # tiles 2c, 2c+1 (c >= 1)
                    lst = []
                    for k in range(KT):
                        xk = xp.tile([128, 1024], MDT, name="xk", tag="xk")
                        nc.sync.dma_start(
                            out=xk, in_=xt[k * 128:(k + 1) * 128, c * 1024:(c + 1) * 1024]
                        )
                        lst.append(xk)
                    xviews[2 * c] = [(xk, 0) for xk in lst]
                    xviews[2 * c + 1] = [(xk, 512) for xk in lst]

                load_tile_split(0)
                # pmat off the critical first HWDGE slots; needed at ~15us
                nc.scalar.dma_start(out=pmat_sb, in_=pmat)
                load_tile_split(1)

                for t in range(TT):
                    if t % 2 == 0 and t + 2 < TT:
                        load_chunk(t // 2 + 1)
                    if t == 2:  # wo is not needed until phase B
                        nc.sync.dma_start(
                            out=wo_sb, in_=woc.rearrange("(h p) n -> p h n", p=128)
                        )
                    # four single-bank PSUM accumulators (q0, q1, k, v) on a
                    # 3-buf rotation
                    slabs = []
                    for m in range(4):
                        ps = psA.tile([128, 512], F32, name="ps_qkv", tag="ps_qkv")
                        for k in range(KT):
                            xk, base = xviews[t][k]
                            nc.tensor.matmul(
                                ps,
                                lhsT=(wqkv_sb[:, k, m * 128:(m + 1) * 128]),
                                rhs=(xk[:, base:base + 512]),
                                start=(k == 0),
                                stop=(k == KT - 1),
                            )
                        slabs.append(ps)

                    s0 = (t % QT_PER_B) * 512  # position-in-sequence
                    for m in range(3):  # q0, q1, k
                        ti = 0 if m < 2 else 2  # cos table index (q vs k)
                        cosT = tab_sb[:, ti, s0:s0 + 512]
                        sinT = tab_sb[:, ti + 1, s0:s0 + 512]
                        src = slabs[m]
                        qk = wp.tile([128, 512], MDT, name="qk", tag="qk")
                        nc.scalar.copy(qk, src)  # PSUM eviction (ACT)
                        sq = wp.tile([128, 512], MDT, name="sq", tag="sq")
                        nc.vector.tensor_mul(sq, qk, qk)
                        nc.gpsimd.partition_all_reduce(sq, sq, 128, ReduceOp.add)
                        # rstd in every partition (allreduce output is
                        # replicated): no broadcast needed
                        rstd = wp.tile([128, 512], MDT, name="rstd", tag="rstd")
                        if USE_ARS:
                            nc.scalar.activation(
                                rstd, sq,
                                mybir.ActivationFunctionType.Abs_reciprocal_sqrt,
                                bias=eps_col, scale=1.0 / D,
                            )
                        else:
                            rr = wp.tile([128, 512], F32, name="rr", tag="rr")
                            nc.scalar.activation(
                                rr, sq, mybir.ActivationFunctionType.Sqrt,
                                bias=eps_col, scale=1.0 / D,
                            )
                            with nc.allow_low_precision(reason="rstd bf16"):
                                nc.vector.reciprocal(rstd, rr)
                        shf = psR.tile([128, 512], F32, name="shf", tag="shf")
                        nc.tensor.matmul(shf, lhsT=pmat_sb, rhs=qk, start=True, stop=True)
                        t0 = wp.tile([128, 512], MDT, name="t0", tag="t0")
                        nc.vector.tensor_mul(t0, qk, cosT)
                        t1 = wp.tile([128, 512], MDT, name="t1", tag="t1")
                        nc.vector.tensor_mul(t1, shf, sinT)  # reads PSUM
                        tr = wp.tile([128, 512], MDT, name="tr", tag="tr")
                        nc.vector.tensor_add(tr, t0, t1)
                        if m < 2:
                            dst = qt_sb[:, m, t * 512:(t + 1) * 512]
                        else:
                            dst = kt_sb[:, t * 512:(t + 1) * 512]
                        nc.vector.tensor_mul(dst, tr, rstd)
                    # V: evict transposed VT (bf16) then DMA-xbar transpose
                    # to natural [tok, d]
                    vt = wp.tile([128, 512], MDT, name="vt", tag="vt")
                    nc.scalar.copy(vt, slabs[3])
                    nc.sync.dma_start_transpose(
                        v_sb[:, t * 4:(t + 1) * 4, :], vt
                    )
                    if t == 4:
                        # (b0,qt0)'s attention score stage rides phase A's
                        # tail (its K/Q inputs completed with tile 3)
                        prestates[0] = [
                            slice_scores(0, 0, h, qh) for h, qh in b_slices]

            # ---------------- Phase B: causal attention + o-proj --------------
            with ExitStack() as pb:
                atp = pb.enter_context(tc.tile_pool(name="atp", bufs=8))
                op = pb.enter_context(tc.tile_pool(name="op", bufs=3))
                psP = pb.enter_context(tc.tile_pool(name="psP", bufs=2, space="PSUM"))

                def emit_oproj_mq(b, q0, at_tiles, mq, fine=False):
                    # one 128-row block of the o-proj partial for rows
                    # [b*S+q0, +512): two-bank po pairs so each eviction
                    # moves [128, 1024] (eviction is the o-proj bottleneck
                    # at [128, 512] granularity); one bf16 DMA per block.
                    # fine=True (very last block): halves evicted on DVE+ACT
                    # in parallel and two half-DMAs, shortening the
                    # end-of-program drain chain
                    qh = mq // 2
                    mq2 = mq % 2  # 128-slice within the 256 at tile
                    ob4 = op.tile([128, 4, 512], MDT, name="ob4", tag="ob4")
                    for np_ in range(2):
                        po = psP.tile([128, 2, 512], F32, name="po", tag="po")
                        for j in range(2):
                            nn = 2 * np_ + j
                            for h in range(HQ):
                                nc.tensor.matmul(
                                    po[:, j, :],
                                    lhsT=(at_tiles[(h, qh)][:, mq2 * 128:(mq2 + 1) * 128]),
                                    rhs=(wo_sb[:, h, nn * 512:(nn + 1) * 512]),
                                    start=(h == 0), stop=(h == HQ - 1),
                                )
                        dst = ob4[:, 2 * np_:2 * np_ + 2, :]
                        # GPSIMD cannot read PSUM on HW: DVE/ACT alternate
                        if fine:
                            nc.vector.tensor_copy(ob4[:, 2 * np_, :], po[:, 0, :])
                            nc.scalar.copy(ob4[:, 2 * np_ + 1, :], po[:, 1, :])
                            nc.sync.dma_start(
                                out=out[b * S + q0 + mq * 128:
                                        b * S + q0 + (mq + 1) * 128,
                                        np_ * 1024:(np_ + 1) * 1024],
                                in_=dst,
                            )
                        elif (mq * 2 + np_) % 2 == 0:
                            nc.vector.tensor_copy(dst, po)
                        else:
                            nc.scalar.copy(dst, po)
                    if not fine:
                        nc.sync.dma_start(
                            out=out[b * S + q0 + mq * 128: b * S + q0 + (mq + 1) * 128, :],
                            in_=ob4,
                        )

                def slice_pv(state, at_tiles):
                    # PV accumulation over k-tiles, then normalize. The last
                    # k-tile sits above the diagonal for the lower 128 q
                    # columns (masked to zero), so its matmul runs at half
                    # free-width; emitted last with stop=True (PE executes
                    # in order, so the series is complete at the stop).
                    b, h, qh, n_kt, ets, rb = state
                    ot = psO_t[:, pv_count[0] % 2, :]
                    pv_count[0] += 1
                    for kt in range(n_kt - 1):
                        nc.tensor.matmul(
                            ot, lhsT=(v_sb[:, b * (S // 128) + kt, :]),
                            rhs=(ets[kt]),
                            start=(kt == 0), stop=False,
                        )
                    nc.tensor.matmul(
                        ot[:, 128:256],
                        lhsT=(v_sb[:, b * (S // 128) + n_kt - 1, :]),
                        rhs=(ets[n_kt - 1][:, 128:256]),
                        start=False, stop=True,
                    )
                    at = atp.tile([128, 256], MDT, name="at", tag="at")
                    nc.vector.tensor_mul(at, ot, rb)
                    at_tiles[(h, qh)] = at

                # o-proj runs one q-tile behind the attention slices so the
                # PE never waits on the softmax-denominator tail
                pending = None
                for b in range(B):
                    for qt in range(QT_PER_B):
                        q0 = qt * 512
                        at_tiles = {}
                        if b == 0 and qt in prestates:
                            # score stage already emitted in phase A's tail
                            for st in prestates[qt]:
                                slice_pv(st, at_tiles)
                        else:
                            # deferred o-proj emitted as two half-blocks
                            # between complete slices: the intervening
                            # score/PV matmuls drain the psP pair rotation
                            for i, (h, qh) in enumerate(b_slices):
                                st = slice_scores(b, q0, h, qh)
                                slice_pv(st, at_tiles)
                                if pending is not None and i in (1, 3):
                                    emit_oproj_mq(*pending, i - 1)
                                    emit_oproj_mq(*pending, i)
                        if pending is not None and b == 0 and qt in prestates:
                            for mq in range(4):
                                emit_oproj_mq(*pending, mq)
                        pending = (b, q0, at_tiles)
                for mq in range(4):
                    emit_oproj_mq(*pending, mq, fine=(mq == 3))
    nc.compile()
    return nc


def _rot_half(w):
    return np.concatenate([w[D // 2:], w[:D // 2]])


def prep_inputs(x, cos, sin, wq, wk, wv, wo, q_norm_w, k_norm_w):
    """Host-side sharding/layout prep. Returns per-core in_maps."""
    import ml_dtypes
    f = np.float32
    mf = np.dtype(ml_dtypes.bfloat16)
    cvt = lambda a: np.ascontiguousarray(a.astype(mf))
    x = np.asarray(x, f)
    cos = np.asarray(cos, f)
    sin = np.asarray(sin, f)
    wq, wk, wv, wo = (np.asarray(a, f) for a in (wq, wk, wv, wo))
    q_norm_w = np.asarray(q_norm_w, f)
    k_norm_w = np.asarray(k_norm_w, f)

    xt = np.ascontiguousarray(x.reshape(T, HID).T)  # [HID, T]
    ctq = cos.T * q_norm_w[:, None]
    stq = sin.T * _rot_half(q_norm_w)[:, None]
    ctk = cos.T * k_norm_w[:, None]
    stk = sin.T * _rot_half(k_norm_w)[:, None]
    tab4 = np.stack([ctq, stq, ctk, stk], axis=1)  # [D, 4, S]
    # rotate-half permutation (with sign) as a matmul stationary operand:
    # out[d] = sum_j pmat[j, d] * q[j] = sign(d) * q[(d+64) % 128]
    pmat = np.zeros((D, D), f)
    for d in range(D // 2):
        pmat[d + D // 2, d] = -1.0
    for d in range(D // 2, D):
        pmat[d - D // 2, d] = 1.0
    xt_m, tab4_m, pmat_m = cvt(xt), cvt(tab4), cvt(pmat)

    in_maps = []
    for c in range(NCORES):
        wqkv_c = np.ascontiguousarray(np.concatenate([
            wq[:, c * HQ * D:(c + 1) * HQ * D],
            wk[:, c * D:(c + 1) * D],
            wv[:, c * D:(c + 1) * D],
        ], axis=1))
        woc = np.ascontiguousarray(wo[c * HQ * D:(c + 1) * HQ * D, :])
        in_maps.append({
            "xt": xt_m, "wqkv": cvt(wqkv_c), "woc": cvt(woc),
            "pmat": pmat_m, "tab4": tab4_m,
        })
    return in_maps


_NC = None


def get_nc():
    global _NC
    if _NC is None:
        _NC = build_nc()
    return _NC


def kernel(x, cos, sin, wq, wk, wv, wo, q_norm_w, k_norm_w):
    nc = get_nc()
    in_maps = prep_inputs(x, cos, sin, wq, wk, wv, wo, q_norm_w, k_norm_w)
    res = run_bass_kernel_spmd(nc, in_maps, core_ids=list(range(NCORES)))
    acc = np.zeros((T, HID), dtype=np.float64)
    for c in range(NCORES):
        acc += res.results[c]["out"]
    return acc.astype(np.float32).reshape(B, S, HID)
